# revision 40
# baseline (speedup 1.0000x reference)
"""Multi-head self-attention (B=2, T=2048, D=2048, H=16, RoPE, causal)
as a Bass/Tile kernel running SPMD on 8 trn2 NeuronCores.

Sharding: tensor-parallel over heads (2 heads per core). Each core
computes its heads' Q/K/V projections, RoPE, causal attention, and a
partial out-projection over its 256 feature columns; the host sums the
8 partial outputs (all-reduce equivalent).

Dataflow (per core, per batch):
  - x streamed per 512-wide t-block ([128, 16, 512] SBUF tiles, 4 tags);
    the first block's DMA is interleaved per-contraction-chunk with the
    weight loads so the PE starts ~2us in.
  - Q/K projections in "T-layout" (feature dim on partitions, time on
    free); RoPE rotate-half via a PE permutation matmul, combines on DVE
    in bf16 (2x mode where operands allow).
  - V projected directly in natural layout ([tk, d]): lhsT = x chunk,
    rhs = Wv slice -- no PE transposes.
  - scores computed transposed: S^T[tk, tq] per (key-chunk, q-group).
    Chunks are narrowed to the causal region (exact 136-block lower
    triangle, no fully-masked work); only the diagonal 128x128 block
    gets a mask add. The two heads' chunk streams are interleaved so
    the PE always has ~1.3us of work while exp round-trips through
    DVE/Act. Z row sums via a [128,1] ones matmul accumulated in PSUM.
  - normalization trails each q-group: po -> oT (unnormalized cast),
    1/Z table via DVE reciprocal, then a ones-row broadcast matmul
    (riding the po PSUM slots between groups) + in-place DVE multiply.
  - out-projection accumulates the two head-chunks in PSUM; partial
    result cast to f16 and DMA'd out; host sums partials across cores.
"""

import sys

sys.path.insert(0, "/opt/trn_rl_repo")

import ml_dtypes
import numpy as np

import concourse.bass as bass
import concourse.mybir as mybir
import concourse.tile as tile
from concourse.bass_utils import run_bass_kernel_spmd


def _legalize_waits(nc):
    """Walrus codegen rejects >2 sync waits on DMA/matmul/nop-class
    instructions, and Tile's pool-recycle waits bypass its own elision.
    Spill excess waits (>1) onto freshly inserted same-engine NoOps
    placed immediately before the offending instruction (sound w.r.t.
    per-engine program order)."""
    spill_id = [0]
    for bb in nc.m.functions[0].blocks:
        new_insts = []
        for inst in bb.instructions:
            si = getattr(inst, "sync_info", None)
            if si is None or not si.on_wait:
                new_insts.append(inst)
                continue
            eng = getattr(inst, "engine", None)
            kept = list(si.on_wait)
            if len(kept) > 1 and eng is not None:
                excess, kept = kept[:-1], kept[-1:]
                for w in excess:
                    spill_id[0] += 1
                    nop = mybir.InstNoOp(
                        name=f"I-wspill-{spill_id[0]}",
                        ins=[],
                        outs=[],
                        engine=eng,
                    )
                    nop.sync_info = mybir.SyncInfo(on_wait=[w], on_update=[])
                    new_insts.append(nop)
            if len(kept) != len(si.on_wait):
                si.on_wait[:] = kept
            new_insts.append(inst)
        if len(new_insts) != len(bb.instructions):
            bb.instructions[:] = new_insts


_PHASE_MARKS = []  # (phase_label, last_inst_index_before_phase) - profiling aid


def _mark(nc, label):
    n = -1
    for fn in nc.m.functions:
        for bb in fn.blocks:
            for ins in bb.instructions:
                if ins.name.startswith("I-"):
                    try:
                        n = max(n, int(ins.name[2:]))
                    except ValueError:
                        pass
    _PHASE_MARKS.append((label, n))


B, T, D, H, HD = 2, 2048, 2048, 16, 128
NCORES = 8
HPC = H // NCORES            # heads per core = 2
M_PC = HPC * HD              # per-core feature slice = 256
BT = B * T                   # 4096
SCALE = HD ** -0.5
ROPE_THETA = 10000.0

F32 = mybir.dt.float32
F16 = mybir.dt.float16
BF16 = mybir.dt.bfloat16
BF16_NP = ml_dtypes.bfloat16

TB = 512                     # t-block for projections / q-groups
NTB_B = T // TB              # 4 t-blocks per batch
NMC = D // 128               # 16 contraction chunks
NKC = T // 128               # 16 key chunks per batch
JPG = TB // 128              # key chunks per q-group width = 4

Copy = mybir.ActivationFunctionType.Copy
Exp = mybir.ActivationFunctionType.Exp


def build_program():
    nc = bass.Bass()

    xT_d = nc.declare_dram_parameter("xT", [D, BT], BF16, isOutput=False)
    perm_d = nc.declare_dram_parameter("permM", [HD, HD], BF16, isOutput=False)
    negm_d = nc.declare_dram_parameter("negmM", [128, 128], F32, isOutput=False)
    # wq and wk concatenated so one DMA covers both (halves SP-seq time
    # on the critical startup path)
    wqk_d = nc.declare_dram_parameter(
        "wqkT", [D, 2 * M_PC], BF16, isOutput=False
    )
    wv_d = nc.declare_dram_parameter("wvT", [D, M_PC], BF16, isOutput=False)
    wo_d = nc.declare_dram_parameter("woT", [M_PC, D], BF16, isOutput=False)
    cos_d = nc.declare_dram_parameter("cosT", [HD, T], BF16, isOutput=False)
    sinh_d = nc.declare_dram_parameter("sinhT", [HD, T], BF16, isOutput=False)
    out_d = nc.declare_dram_parameter("partialT", [D, BT], F16, isOutput=True)

    xT_v = xT_d.rearrange("(c p) t -> p c t", p=128)      # [128, 16, BT]
    wqk_v = wqk_d.rearrange("(c p) n -> p c n", p=128)    # [128, 16, 512]
    wv_v = wv_d.rearrange("(c p) n -> p c n", p=128)
    wo_v = wo_d.rearrange("(c p) n -> p c n", p=128)      # [128, 2, 2048]
    out_v = out_d.rearrange("(c p) t -> p c t", p=128)    # [128, 16, BT]

    with tile.TileContext(nc) as tc:
        with (
            tc.tile_pool(name="wpool", bufs=1) as wpool,
            tc.tile_pool(name="xp", bufs=1) as xp,
            tc.tile_pool(name="big", bufs=1) as big,
            tc.tile_pool(name="rp", bufs=2) as rp,
            tc.tile_pool(name="attn_sb", bufs=6) as asb,
            tc.tile_pool(name="fs_sb", bufs=3) as fsb,
        ):
            # ---- weights + first x block, interleaved in graduated mc
            # groups (fast pipeline fill, then few big SP-cheap DMAs) ----
            wqk_sb = wpool.tile([128, NMC, 2 * M_PC], BF16, tag="wqk")
            wv_sb = wpool.tile([128, NMC, M_PC], BF16, tag="wv")
            x_tiles = {}
            xt0 = xp.tile([128, NMC, TB], BF16, tag="x0", name="x_b0_t0")
            x_tiles[(0, 0)] = xt0
            for lo, hi in ((0, 1), (1, 2), (2, 3), (3, 4), (4, 6), (6, 8),
                           (8, 10), (10, 12), (12, 14), (14, 16)):
                nc.sync.dma_start(
                    out=wqk_sb[:, lo:hi, :], in_=wqk_v[:, lo:hi, :]
                )
                # first x chunk rides the idle DVE queue, in parallel with
                # SP's weight DMA, to cut the cold-start latency
                eng = nc.scalar if lo == 0 else nc.sync
                eng.dma_start(
                    out=xt0[:, lo:hi, :], in_=xT_v[:, lo:hi, 0:TB]
                )

            cos_sb = wpool.tile([128, T], BF16, tag="cos")
            sinh_sb = wpool.tile([128, T], BF16, tag="sinh")
            perm_sb = wpool.tile([HD, HD], BF16, tag="perm")
            nc.sync.dma_start(out=perm_sb, in_=perm_d[:, :])
            nc.sync.dma_start(out=cos_sb[:, 0:TB], in_=cos_d[:, 0:TB])
            nc.sync.dma_start(out=sinh_sb[:, 0:TB], in_=sinh_d[:, 0:TB])

            def load_x(b, tb):
                t = xp.tile(
                    [128, NMC, TB], BF16, tag=f"x{tb}", name=f"x_b{b}_t{tb}"
                )
                x_tiles[(b, tb)] = t
                lo = b * T + tb * TB
                for m0 in range(0, NMC, 4):
                    nc.sync.dma_start(
                        out=t[:, m0 : m0 + 4, :],
                        in_=xT_v[:, m0 : m0 + 4, lo : lo + TB],
                    )

            # wv rides alongside tb0's V matmuls; x block 1 follows
            for m0 in range(0, NMC, 4):
                nc.sync.dma_start(
                    out=wv_sb[:, m0 : m0 + 4, :], in_=wv_v[:, m0 : m0 + 4, :]
                )
            load_x(0, 1)
            nc.sync.dma_start(out=cos_sb[:, TB:], in_=cos_d[:, TB:])
            nc.sync.dma_start(out=sinh_sb[:, TB:], in_=sinh_d[:, TB:])
            negm = wpool.tile([128, 128], F32, tag="negm")
            nc.sync.dma_start(out=negm, in_=negm_d[:, :])
            ones_col = wpool.tile([128, 1], BF16, tag="ones_c")
            nc.vector.memset(ones_col, 1.0)
            ones_row = wpool.tile([1, 128], BF16, tag="ones_r")
            nc.vector.memset(ones_row, 1.0)
            # 1/Z table: [1, HPC*T], column h*T + t (kept on partition 0)
            zrs_tab = wpool.tile([1, HPC * T], BF16, tag="zrs")

            for tb in range(2, NTB_B):
                load_x(0, tb)

            wo_sb = wpool.tile([128, HPC, D], BF16, tag="wo")
            nc.sync.dma_start(out=wo_sb, in_=wo_v)

            for b in range(B):
                t0 = b * T  # global t offset of this batch
                _mark(nc, f"b{b}_proj")

                # persistent per-batch tensors (slots reused across b)
                qT = big.tile([128, HPC, T], BF16, tag="qT")   # [hd, h, t]
                kT = big.tile([128, HPC, T], BF16, tag="kT")
                vN = big.tile([128, NKC, M_PC], BF16, tag="vN")  # [tk, j, n]
                oT = big.tile([128, HPC, T], BF16, tag="oT")   # attn out

                # ---------------- projections + RoPE ----------------
                with tc.tile_pool(name="qk_ps", bufs=1, space="PSUM") as qkp, \
                     tc.tile_pool(name="v_ps", bufs=1, space="PSUM") as vps:
                    for tb in range(NTB_B):
                        xt = x_tiles[(b, tb)]
                        ts_l = slice(tb * TB, (tb + 1) * TB)   # in-batch
                        ps = {}
                        for h in range(HPC):
                            for nm in ("q", "k"):
                                ps[nm, h] = qkp.tile(
                                    [128, TB], F32, tag=f"{nm}{h}",
                                    name=f"ps_{nm}{h}",
                                )
                        for mc in range(NMC):
                            for h in range(HPC):
                                for ni, nm in ((0, "q"), (1, "k")):
                                    hs = slice(
                                        ni * M_PC + h * HD,
                                        ni * M_PC + (h + 1) * HD,
                                    )
                                    nc.tensor.matmul(
                                        ps[nm, h],
                                        lhsT=wqk_sb[:, mc, hs],
                                        rhs=xt[:, mc, :],
                                        start=(mc == 0),
                                        stop=(mc == NMC - 1),
                                    )
                        # raw bf16 casts early on Act (deps ready now)
                        raws = {}
                        for nm in ("q", "k"):
                            for h in range(HPC):
                                raw = rp.tile(
                                    [128, TB], BF16, tag=f"raw{nm}{h}"
                                )
                                nc.scalar.activation(raw, ps[nm, h], Copy)
                                raws[nm, h] = raw
                        # RoPE before V: the qk/sw PSUM banks drain while the
                        # PE streams V matmuls, so the next phase's bank-WAR
                        # waits resolve before the PE gets there
                        for nm, dest in (("q", qT), ("k", kT)):
                            for h in range(HPC):
                                raw = raws[nm, h]
                                swb = rp.tile([128, TB], BF16, tag="swb")
                                nc.sync.dma_start(
                                    out=swb[0:64, :], in_=raw[64:128, :]
                                )
                                nc.sync.dma_start(
                                    out=swb[64:128, :], in_=raw[0:64, :]
                                )
                                t2 = rp.tile([128, TB], BF16, tag="t2")
                                nc.vector.tensor_mul(t2, swb, sinh_sb[:, ts_l])
                                t1 = rp.tile([128, TB], BF16, tag="t1")
                                nc.vector.tensor_mul(t1, raw, cos_sb[:, ts_l])
                                nc.vector.tensor_add(dest[:, h, ts_l], t1, t2)
                        # V directly in natural layout: per 128-wide tk chunk
                        for s in range(JPG):
                            j = tb * JPG + s
                            pv = vps.tile(
                                [128, M_PC], F32, tag=f"v{s % 2}",
                                name=f"ps_v{s % 2}",
                            )
                            for mc in range(NMC):
                                nc.tensor.matmul(
                                    pv,
                                    lhsT=xt[:, mc, s * 128 : (s + 1) * 128],
                                    rhs=wv_sb[:, mc, :],
                                    start=(mc == 0),
                                    stop=(mc == NMC - 1),
                                )
                            nc.scalar.activation(vN[:, j, :], pv, Copy)

                if b == 0:
                    # b1's x loads: emitted here so they ride the SP queue
                    # ahead of b0's output stores
                    for tb in range(NTB_B):
                        load_x(1, tb)

                _mark(nc, f"b{b}_attn")
                # ---------------- attention ----------------
                # heads interleaved per key-chunk (both heads' score tiles
                # merged into one 2-bank PSUM tile so a single exp call
                # covers them); Z/PV trail the score/exp stream by one
                # chunk, software-pipelined ACROSS q-groups so the PE never
                # drains at a group boundary. The 1/Z broadcast borrows po
                # slots.
                with tc.tile_pool(name="st_ps", bufs=4, space="PSUM") as stp, \
                     tc.tile_pool(name="pv_ps", bufs=1, space="PSUM") as pvp, \
                     tc.tile_pool(name="z_ps", bufs=1, space="PSUM") as zpp:
                    norm_q = []  # pending (qg, h) normalization steps

                    def norm_step():
                        nqg, h = norm_q.pop(0)
                        qs0 = nqg * TB
                        zbp = pvp.tile(
                            [128, TB], F32, tag=f"po{h}", name="zbp"
                        )
                        nc.tensor.matmul(
                            zbp,
                            lhsT=ones_row,
                            rhs=zrs_tab[0:1, h * T + qs0 : h * T + qs0 + TB],
                            start=True,
                            stop=True,
                        )
                        nc.vector.tensor_mul(
                            oT[:, h, qs0 : qs0 + TB],
                            oT[:, h, qs0 : qs0 + TB],
                            zbp,
                        )
                    # (qg, j) chunk schedule, flattened
                    sched = [
                        (qg, j)
                        for qg in range(NTB_B)
                        for j in range(JPG * (qg + 1))
                    ]
                    po = {}
                    zrow = {}
                    prev = None  # (qg, j, {h: (pt, off)})

                    def zpv_step(pqg, pj, pts):
                        """Z + PV matmuls for the trailing chunk; on the last
                        chunk of a group, also emit recip + po drain."""
                        pjmax = JPG * (pqg + 1)
                        for h in range(HPC):
                            ptp, offp = pts[h]
                            nc.tensor.matmul(
                                zrow[pqg, h][:, offp:],
                                lhsT=ones_col,
                                rhs=ptp[:, offp:],
                                start=(pj == 0),
                                stop=(pj == pjmax - 1),
                            )
                            nc.tensor.matmul(
                                po[pqg, h][:, offp:],
                                lhsT=vN[:, pj, h * HD : (h + 1) * HD],
                                rhs=ptp[:, offp:],
                                start=(pj == 0),
                                stop=(pj == pjmax - 1),
                            )
                        if pj == pjmax - 1:
                            qs0 = pqg * TB
                            last_g = pqg == NTB_B - 1
                            for h in range(HPC):
                                with nc.allow_low_precision(
                                    reason="bf16 1/Z: 0.4% rel, in tolerance"
                                ):
                                    nc.vector.reciprocal(
                                        zrs_tab[
                                            0:1, h * T + qs0 : h * T + qs0 + TB
                                        ],
                                        zrow[pqg, h],
                                    )
                                if last_g:
                                    nc.scalar.activation(
                                        oT[:, h, qs0 : qs0 + TB],
                                        po[pqg, h],
                                        Copy,
                                    )
                                else:
                                    nc.vector.tensor_copy(
                                        oT[:, h, qs0 : qs0 + TB], po[pqg, h]
                                    )
                                norm_q.append((pqg, h))

                    for qg, j in sched:
                        if j == 1:
                            # previous group's 1/Z broadcasts first, so they
                            # take the po-tag slots ahead of this group's po
                            while norm_q:
                                norm_step()
                            for h in range(HPC):
                                po[qg, h] = pvp.tile(
                                    [128, TB], F32, tag=f"po{h}", name=f"po{h}"
                                )
                                zrow[qg, h] = zpp.tile(
                                    [1, TB], F32, tag=f"z{h}", name=f"z{h}"
                                )
                        qs0 = qg * TB
                        off = max(0, (j - JPG * qg) * 128)
                        ks = slice(j * 128, (j + 1) * 128)
                        cur = {}
                        for h in range(HPC):
                            st = stp.tile([128, TB], F32, tag="st")
                            nc.tensor.matmul(
                                st[:, off:],
                                lhsT=kT[:, h, ks],
                                rhs=qT[:, h, qs0 + off : qs0 + TB],
                                start=True,
                                stop=True,
                            )
                            if j >= JPG * qg:  # diagonal 128-block mask
                                nc.vector.tensor_add(
                                    st[:, off : off + 128],
                                    st[:, off : off + 128],
                                    negm,
                                )
                            pt = asb.tile([128, TB], BF16, tag="pt")
                            nc.scalar.activation(
                                pt[:, off:], st[:, off:], Exp, scale=SCALE
                            )
                            cur[h] = (pt, off)
                        if prev is not None:
                            zpv_step(*prev)
                        prev = (qg, j, cur)
                    zpv_step(*prev)
                    while norm_q:
                        norm_step()

                _mark(nc, f"b{b}_outproj")
                # ---------------- out-projection (partial) ----------------
                # output stores grouped 4 column-blocks per DMA (SP-seq time
                # per DMA is the store-path bottleneck, not bandwidth)
                with tc.tile_pool(name="fo_ps", bufs=1, space="PSUM") as fop:
                    for tb in range(NTB_B):
                        tbs = slice(tb * TB, (tb + 1) * TB)
                        fs = None
                        for nb in range(D // 128):
                            nbs = slice(nb * 128, (nb + 1) * 128)
                            fo = fop.tile(
                                [128, TB], F32, tag=f"fo{nb % 4}",
                                name=f"fo{nb % 4}",
                            )
                            for m in range(HPC):
                                nc.tensor.matmul(
                                    fo,
                                    lhsT=wo_sb[:, m, nbs],
                                    rhs=oT[:, m, tbs],
                                    start=(m == 0),
                                    stop=(m == HPC - 1),
                                )
                            grp = (
                                2
                                if (b == B - 1 and tb == NTB_B - 1 and nb >= 12)
                                else 4
                            )
                            if nb % grp == 0:
                                fs = fsb.tile(
                                    [128, 4, TB], F16, tag="fs"
                                )
                            if nb % 2 == 0:
                                nc.vector.tensor_copy(fs[:, nb % grp, :], fo)
                            else:
                                nc.scalar.activation(
                                    fs[:, nb % grp, :], fo, Copy
                                )
                            if nb % grp == grp - 1:
                                last = (
                                    b == B - 1
                                    and tb == NTB_B - 1
                                    and nb == D // 128 - 1
                                )
                                deng = nc.scalar if last else nc.sync
                                deng.dma_start(
                                    out=out_v[
                                        :,
                                        nb - grp + 1 : nb + 1,
                                        t0 + tb * TB : t0 + (tb + 1) * TB,
                                    ],
                                    in_=fs[:, 0:grp, :],
                                )
    _legalize_waits(nc)
    return nc


_NC_CACHE = None


def _get_program():
    global _NC_CACHE
    if _NC_CACHE is None:
        _NC_CACHE = build_program()
    return _NC_CACHE


def _rope_tables():
    inv_freq = 1.0 / (ROPE_THETA ** (np.arange(0, HD, 2, dtype=np.float32) / HD))
    freqs = np.arange(T, dtype=np.float32)[:, None] * inv_freq[None, :]  # (T, 64)
    emb = np.concatenate([freqs, freqs], axis=-1)                        # (T, 128)
    cosT = np.ascontiguousarray(np.cos(emb).T).astype(BF16_NP)           # [128, T]
    sinT = np.sin(emb).T.astype(np.float32)
    sinhT = np.ascontiguousarray(
        np.concatenate([-sinT[: HD // 2], sinT[HD // 2 :]], axis=0)
    ).astype(BF16_NP)
    return cosT, sinhT


def kernel(x, Wq, Wk, Wv, Wo, **run_kwargs):
    x = np.asarray(x, dtype=np.float32)
    Wq = np.asarray(Wq, dtype=np.float32)
    Wk = np.asarray(Wk, dtype=np.float32)
    Wv = np.asarray(Wv, dtype=np.float32)
    Wo = np.asarray(Wo, dtype=np.float32)

    nc = _get_program()
    cosT, sinhT = _rope_tables()
    xT = np.ascontiguousarray(x.reshape(BT, D).T).astype(BF16_NP)  # [D, BT]
    permM = np.zeros((HD, HD), dtype=BF16_NP)
    for m in range(HD):
        permM[(m + HD // 2) % HD, m] = 1.0  # out[m] = in[(m+64)%128]
    # S^T[tk, tq] causal mask for the diagonal block: keep tq(col) >= tk(row)
    r = np.arange(128)
    negmM = np.where(r[None, :] >= r[:, None], 0.0, -1e30).astype(np.float32)

    in_maps = []
    for c in range(NCORES):
        sl = slice(c * M_PC, (c + 1) * M_PC)
        in_maps.append(
            {
                "xT": xT,
                "permM": permM,
                "negmM": negmM,
                "wqkT": np.ascontiguousarray(
                    np.concatenate([Wq[sl, :].T, Wk[sl, :].T], axis=1)
                ).astype(BF16_NP),
                "wvT": np.ascontiguousarray(Wv[sl, :].T).astype(BF16_NP),
                "woT": np.ascontiguousarray(Wo[:, sl].T).astype(BF16_NP),
                "cosT": cosT,
                "sinhT": sinhT,
            }
        )

    res = run_bass_kernel_spmd(nc, in_maps, list(range(NCORES)), **run_kwargs)
    acc = np.zeros((D, BT), dtype=np.float32)
    for c in range(NCORES):
        acc += res.results[c]["partialT"].astype(np.float32)
    out = np.ascontiguousarray(acc.T).reshape(B, T, D)
    if run_kwargs:
        return out, res
    return out


# revision 41
# speedup vs baseline: 1.0011x; 1.0011x over previous
"""Multi-head self-attention (B=2, T=2048, D=2048, H=16, RoPE, causal)
as a Bass/Tile kernel running SPMD on 8 trn2 NeuronCores.

Sharding: tensor-parallel over heads (2 heads per core). Each core
computes its heads' Q/K/V projections, RoPE, causal attention, and a
partial out-projection over its 256 feature columns; the host sums the
8 partial outputs (all-reduce equivalent).

Dataflow (per core, per batch):
  - x streamed per 512-wide t-block ([128, 16, 512] SBUF tiles, 4 tags);
    the first block's DMA is interleaved per-contraction-chunk with the
    weight loads so the PE starts ~2us in.
  - Q/K projections in "T-layout" (feature dim on partitions, time on
    free); RoPE rotate-half via two SBUF->SBUF partition-swap DMAs
    (sign folded into the sin table), all-bf16 combine on DVE (2x mode).
  - V projected directly in natural layout ([tk, d]): lhsT = x chunk,
    rhs = Wv slice -- no PE transposes.
  - scores computed transposed: S^T[tk, tq] per (key-chunk, q-group).
    Chunks are narrowed to the causal region (exact 136-block lower
    triangle, no fully-masked work); only the diagonal 128x128 block
    gets a mask add. The two heads' chunk streams are interleaved so
    the PE always has ~1.3us of work while exp round-trips through
    DVE/Act. Z row sums via a [128,1] ones matmul accumulated in PSUM.
  - normalization trails each q-group: po -> oT (unnormalized cast),
    1/Z table via DVE reciprocal, then a ones-row broadcast matmul
    (riding the po PSUM slots between groups) + in-place DVE multiply.
  - out-projection accumulates the two head-chunks in PSUM; partial
    result cast to f16 and DMA'd out; host sums partials across cores.
"""

import sys

sys.path.insert(0, "/opt/trn_rl_repo")

import ml_dtypes
import numpy as np

import concourse.bass as bass
import concourse.mybir as mybir
import concourse.tile as tile
from concourse.bass_utils import run_bass_kernel_spmd


def _legalize_waits(nc):
    """Walrus codegen rejects >2 sync waits on DMA/matmul/nop-class
    instructions, and Tile's pool-recycle waits bypass its own elision.
    Spill excess waits (>1) onto freshly inserted same-engine NoOps
    placed immediately before the offending instruction (sound w.r.t.
    per-engine program order)."""
    spill_id = [0]
    for bb in nc.m.functions[0].blocks:
        new_insts = []
        for inst in bb.instructions:
            si = getattr(inst, "sync_info", None)
            if si is None or not si.on_wait:
                new_insts.append(inst)
                continue
            eng = getattr(inst, "engine", None)
            kept = list(si.on_wait)
            if len(kept) > 1 and eng is not None:
                excess, kept = kept[:-1], kept[-1:]
                for w in excess:
                    spill_id[0] += 1
                    nop = mybir.InstNoOp(
                        name=f"I-wspill-{spill_id[0]}",
                        ins=[],
                        outs=[],
                        engine=eng,
                    )
                    nop.sync_info = mybir.SyncInfo(on_wait=[w], on_update=[])
                    new_insts.append(nop)
            if len(kept) != len(si.on_wait):
                si.on_wait[:] = kept
            new_insts.append(inst)
        if len(new_insts) != len(bb.instructions):
            bb.instructions[:] = new_insts


_PHASE_MARKS = []  # (phase_label, last_inst_index_before_phase) - profiling aid


def _mark(nc, label):
    n = -1
    for fn in nc.m.functions:
        for bb in fn.blocks:
            for ins in bb.instructions:
                if ins.name.startswith("I-"):
                    try:
                        n = max(n, int(ins.name[2:]))
                    except ValueError:
                        pass
    _PHASE_MARKS.append((label, n))


B, T, D, H, HD = 2, 2048, 2048, 16, 128
NCORES = 8
HPC = H // NCORES            # heads per core = 2
M_PC = HPC * HD              # per-core feature slice = 256
BT = B * T                   # 4096
SCALE = HD ** -0.5
ROPE_THETA = 10000.0

F32 = mybir.dt.float32
F16 = mybir.dt.float16
BF16 = mybir.dt.bfloat16
BF16_NP = ml_dtypes.bfloat16

TB = 512                     # t-block for projections / q-groups
NTB_B = T // TB              # 4 t-blocks per batch
NMC = D // 128               # 16 contraction chunks
NKC = T // 128               # 16 key chunks per batch
JPG = TB // 128              # key chunks per q-group width = 4

Copy = mybir.ActivationFunctionType.Copy
Exp = mybir.ActivationFunctionType.Exp


def build_program():
    nc = bass.Bass()

    xT_d = nc.declare_dram_parameter("xT", [D, BT], BF16, isOutput=False)
    negm_d = nc.declare_dram_parameter("negmM", [128, 128], F32, isOutput=False)
    # wq and wk concatenated so one DMA covers both (halves SP-seq time
    # on the critical startup path)
    wqk_d = nc.declare_dram_parameter(
        "wqkT", [D, 2 * M_PC], BF16, isOutput=False
    )
    wv_d = nc.declare_dram_parameter("wvT", [D, M_PC], BF16, isOutput=False)
    wo_d = nc.declare_dram_parameter("woT", [M_PC, D], BF16, isOutput=False)
    cos_d = nc.declare_dram_parameter("cosT", [HD, T], BF16, isOutput=False)
    sinh_d = nc.declare_dram_parameter("sinhT", [HD, T], BF16, isOutput=False)
    out_d = nc.declare_dram_parameter("partialT", [D, BT], F16, isOutput=True)

    xT_v = xT_d.rearrange("(c p) t -> p c t", p=128)      # [128, 16, BT]
    wqk_v = wqk_d.rearrange("(c p) n -> p c n", p=128)    # [128, 16, 512]
    wv_v = wv_d.rearrange("(c p) n -> p c n", p=128)
    wo_v = wo_d.rearrange("(c p) n -> p c n", p=128)      # [128, 2, 2048]
    out_v = out_d.rearrange("(c p) t -> p c t", p=128)    # [128, 16, BT]

    with tile.TileContext(nc) as tc:
        with (
            tc.tile_pool(name="wpool", bufs=1) as wpool,
            tc.tile_pool(name="xp", bufs=1) as xp,
            tc.tile_pool(name="big", bufs=1) as big,
            tc.tile_pool(name="rp", bufs=2) as rp,
            tc.tile_pool(name="attn_sb", bufs=6) as asb,
            tc.tile_pool(name="fs_sb", bufs=3) as fsb,
        ):
            # ---- weights + first x block, interleaved in graduated mc
            # groups (fast pipeline fill, then few big SP-cheap DMAs) ----
            wqk_sb = wpool.tile([128, NMC, 2 * M_PC], BF16, tag="wqk")
            wv_sb = wpool.tile([128, NMC, M_PC], BF16, tag="wv")
            x_tiles = {}
            xt0 = xp.tile([128, NMC, TB], BF16, tag="x0", name="x_b0_t0")
            x_tiles[(0, 0)] = xt0
            for lo, hi in ((0, 1), (1, 2), (2, 3), (3, 4), (4, 6), (6, 8),
                           (8, 10), (10, 12), (12, 14), (14, 16)):
                nc.sync.dma_start(
                    out=wqk_sb[:, lo:hi, :], in_=wqk_v[:, lo:hi, :]
                )
                # first x chunk rides the idle DVE queue, in parallel with
                # SP's weight DMA, to cut the cold-start latency
                eng = nc.scalar if lo == 0 else nc.sync
                eng.dma_start(
                    out=xt0[:, lo:hi, :], in_=xT_v[:, lo:hi, 0:TB]
                )

            cos_sb = wpool.tile([128, T], BF16, tag="cos")
            sinh_sb = wpool.tile([128, T], BF16, tag="sinh")
            nc.sync.dma_start(out=cos_sb[:, 0:TB], in_=cos_d[:, 0:TB])
            nc.sync.dma_start(out=sinh_sb[:, 0:TB], in_=sinh_d[:, 0:TB])

            def load_x(b, tb):
                t = xp.tile(
                    [128, NMC, TB], BF16, tag=f"x{tb}", name=f"x_b{b}_t{tb}"
                )
                x_tiles[(b, tb)] = t
                lo = b * T + tb * TB
                for m0 in range(0, NMC, 4):
                    nc.sync.dma_start(
                        out=t[:, m0 : m0 + 4, :],
                        in_=xT_v[:, m0 : m0 + 4, lo : lo + TB],
                    )

            # wv rides alongside tb0's V matmuls; x block 1 follows
            for m0 in range(0, NMC, 4):
                nc.sync.dma_start(
                    out=wv_sb[:, m0 : m0 + 4, :], in_=wv_v[:, m0 : m0 + 4, :]
                )
            load_x(0, 1)
            nc.sync.dma_start(out=cos_sb[:, TB:], in_=cos_d[:, TB:])
            nc.sync.dma_start(out=sinh_sb[:, TB:], in_=sinh_d[:, TB:])
            negm = wpool.tile([128, 128], F32, tag="negm")
            nc.sync.dma_start(out=negm, in_=negm_d[:, :])
            ones_col = wpool.tile([128, 1], BF16, tag="ones_c")
            nc.vector.memset(ones_col, 1.0)
            ones_row = wpool.tile([1, 128], BF16, tag="ones_r")
            nc.vector.memset(ones_row, 1.0)
            # 1/Z table: [1, HPC*T], column h*T + t (kept on partition 0)
            zrs_tab = wpool.tile([1, HPC * T], BF16, tag="zrs")

            for tb in range(2, NTB_B):
                load_x(0, tb)

            wo_sb = wpool.tile([128, HPC, D], BF16, tag="wo")
            nc.sync.dma_start(out=wo_sb, in_=wo_v)

            for b in range(B):
                t0 = b * T  # global t offset of this batch
                _mark(nc, f"b{b}_proj")

                # persistent per-batch tensors (slots reused across b)
                qT = big.tile([128, HPC, T], BF16, tag="qT")   # [hd, h, t]
                kT = big.tile([128, HPC, T], BF16, tag="kT")
                vN = big.tile([128, NKC, M_PC], BF16, tag="vN")  # [tk, j, n]
                oT = big.tile([128, HPC, T], BF16, tag="oT")   # attn out

                # ---------------- projections + RoPE ----------------
                with tc.tile_pool(name="qk_ps", bufs=1, space="PSUM") as qkp, \
                     tc.tile_pool(name="v_ps", bufs=1, space="PSUM") as vps:
                    for tb in range(NTB_B):
                        xt = x_tiles[(b, tb)]
                        ts_l = slice(tb * TB, (tb + 1) * TB)   # in-batch
                        ps = {}
                        for h in range(HPC):
                            for nm in ("q", "k"):
                                ps[nm, h] = qkp.tile(
                                    [128, TB], F32, tag=f"{nm}{h}",
                                    name=f"ps_{nm}{h}",
                                )
                        for mc in range(NMC):
                            for h in range(HPC):
                                for ni, nm in ((0, "q"), (1, "k")):
                                    hs = slice(
                                        ni * M_PC + h * HD,
                                        ni * M_PC + (h + 1) * HD,
                                    )
                                    nc.tensor.matmul(
                                        ps[nm, h],
                                        lhsT=wqk_sb[:, mc, hs],
                                        rhs=xt[:, mc, :],
                                        start=(mc == 0),
                                        stop=(mc == NMC - 1),
                                    )
                        # raw bf16 casts early on Act (deps ready now)
                        raws = {}
                        for nm in ("q", "k"):
                            for h in range(HPC):
                                raw = rp.tile(
                                    [128, TB], BF16, tag=f"raw{nm}{h}"
                                )
                                nc.scalar.activation(raw, ps[nm, h], Copy)
                                raws[nm, h] = raw
                        # RoPE before V: the qk/sw PSUM banks drain while the
                        # PE streams V matmuls, so the next phase's bank-WAR
                        # waits resolve before the PE gets there
                        for nm, dest in (("q", qT), ("k", kT)):
                            for h in range(HPC):
                                raw = raws[nm, h]
                                swb = rp.tile([128, TB], BF16, tag="swb")
                                nc.sync.dma_start(
                                    out=swb[0:64, :], in_=raw[64:128, :]
                                )
                                nc.sync.dma_start(
                                    out=swb[64:128, :], in_=raw[0:64, :]
                                )
                                t2 = rp.tile([128, TB], BF16, tag="t2")
                                nc.vector.tensor_mul(t2, swb, sinh_sb[:, ts_l])
                                t1 = rp.tile([128, TB], BF16, tag="t1")
                                nc.vector.tensor_mul(t1, raw, cos_sb[:, ts_l])
                                nc.vector.tensor_add(dest[:, h, ts_l], t1, t2)
                        # V directly in natural layout: per 128-wide tk chunk
                        for s in range(JPG):
                            j = tb * JPG + s
                            pv = vps.tile(
                                [128, M_PC], F32, tag=f"v{s % 2}",
                                name=f"ps_v{s % 2}",
                            )
                            for mc in range(NMC):
                                nc.tensor.matmul(
                                    pv,
                                    lhsT=xt[:, mc, s * 128 : (s + 1) * 128],
                                    rhs=wv_sb[:, mc, :],
                                    start=(mc == 0),
                                    stop=(mc == NMC - 1),
                                )
                            nc.scalar.activation(vN[:, j, :], pv, Copy)

                if b == 0:
                    # b1's x loads: emitted here so they ride the SP queue
                    # ahead of b0's output stores
                    for tb in range(NTB_B):
                        load_x(1, tb)

                _mark(nc, f"b{b}_attn")
                # ---------------- attention ----------------
                # heads interleaved per key-chunk (both heads' score tiles
                # merged into one 2-bank PSUM tile so a single exp call
                # covers them); Z/PV trail the score/exp stream by one
                # chunk, software-pipelined ACROSS q-groups so the PE never
                # drains at a group boundary. The 1/Z broadcast borrows po
                # slots.
                with tc.tile_pool(name="st_ps", bufs=4, space="PSUM") as stp, \
                     tc.tile_pool(name="pv_ps", bufs=1, space="PSUM") as pvp, \
                     tc.tile_pool(name="z_ps", bufs=1, space="PSUM") as zpp:
                    norm_q = []  # pending (qg, h) normalization steps

                    def norm_step():
                        nqg, h = norm_q.pop(0)
                        qs0 = nqg * TB
                        zbp = pvp.tile(
                            [128, TB], F32, tag=f"po{h}", name="zbp"
                        )
                        nc.tensor.matmul(
                            zbp,
                            lhsT=ones_row,
                            rhs=zrs_tab[0:1, h * T + qs0 : h * T + qs0 + TB],
                            start=True,
                            stop=True,
                        )
                        nc.vector.tensor_mul(
                            oT[:, h, qs0 : qs0 + TB],
                            oT[:, h, qs0 : qs0 + TB],
                            zbp,
                        )
                    # (qg, j) chunk schedule, flattened
                    sched = [
                        (qg, j)
                        for qg in range(NTB_B)
                        for j in range(JPG * (qg + 1))
                    ]
                    po = {}
                    zrow = {}
                    prev = None  # (qg, j, {h: (pt, off)})

                    def zpv_step(pqg, pj, pts):
                        """Z + PV matmuls for the trailing chunk; on the last
                        chunk of a group, also emit recip + po drain."""
                        pjmax = JPG * (pqg + 1)
                        for h in range(HPC):
                            ptp, offp = pts[h]
                            nc.tensor.matmul(
                                zrow[pqg, h][:, offp:],
                                lhsT=ones_col,
                                rhs=ptp[:, offp:],
                                start=(pj == 0),
                                stop=(pj == pjmax - 1),
                            )
                            nc.tensor.matmul(
                                po[pqg, h][:, offp:],
                                lhsT=vN[:, pj, h * HD : (h + 1) * HD],
                                rhs=ptp[:, offp:],
                                start=(pj == 0),
                                stop=(pj == pjmax - 1),
                            )
                        if pj == pjmax - 1:
                            qs0 = pqg * TB
                            last_g = pqg == NTB_B - 1
                            for h in range(HPC):
                                with nc.allow_low_precision(
                                    reason="bf16 1/Z: 0.4% rel, in tolerance"
                                ):
                                    nc.vector.reciprocal(
                                        zrs_tab[
                                            0:1, h * T + qs0 : h * T + qs0 + TB
                                        ],
                                        zrow[pqg, h],
                                    )
                                if last_g:
                                    nc.scalar.activation(
                                        oT[:, h, qs0 : qs0 + TB],
                                        po[pqg, h],
                                        Copy,
                                    )
                                else:
                                    nc.vector.tensor_copy(
                                        oT[:, h, qs0 : qs0 + TB], po[pqg, h]
                                    )
                                norm_q.append((pqg, h))

                    for qg, j in sched:
                        if j == 1:
                            # previous group's 1/Z broadcasts first, so they
                            # take the po-tag slots ahead of this group's po
                            while norm_q:
                                norm_step()
                            for h in range(HPC):
                                po[qg, h] = pvp.tile(
                                    [128, TB], F32, tag=f"po{h}", name=f"po{h}"
                                )
                                zrow[qg, h] = zpp.tile(
                                    [1, TB], F32, tag=f"z{h}", name=f"z{h}"
                                )
                        qs0 = qg * TB
                        off = max(0, (j - JPG * qg) * 128)
                        ks = slice(j * 128, (j + 1) * 128)
                        cur = {}
                        for h in range(HPC):
                            st = stp.tile([128, TB], F32, tag="st")
                            nc.tensor.matmul(
                                st[:, off:],
                                lhsT=kT[:, h, ks],
                                rhs=qT[:, h, qs0 + off : qs0 + TB],
                                start=True,
                                stop=True,
                            )
                            if j >= JPG * qg:  # diagonal 128-block mask
                                nc.vector.tensor_add(
                                    st[:, off : off + 128],
                                    st[:, off : off + 128],
                                    negm,
                                )
                            pt = asb.tile([128, TB], BF16, tag="pt")
                            nc.scalar.activation(
                                pt[:, off:], st[:, off:], Exp, scale=SCALE
                            )
                            cur[h] = (pt, off)
                        if prev is not None:
                            zpv_step(*prev)
                        prev = (qg, j, cur)
                    zpv_step(*prev)
                    while norm_q:
                        norm_step()

                _mark(nc, f"b{b}_outproj")
                # ---------------- out-projection (partial) ----------------
                # output stores grouped 4 column-blocks per DMA (SP-seq time
                # per DMA is the store-path bottleneck, not bandwidth)
                with tc.tile_pool(name="fo_ps", bufs=1, space="PSUM") as fop:
                    for tb in range(NTB_B):
                        tbs = slice(tb * TB, (tb + 1) * TB)
                        fs = None
                        for nb in range(D // 128):
                            nbs = slice(nb * 128, (nb + 1) * 128)
                            fo = fop.tile(
                                [128, TB], F32, tag=f"fo{nb % 4}",
                                name=f"fo{nb % 4}",
                            )
                            for m in range(HPC):
                                nc.tensor.matmul(
                                    fo,
                                    lhsT=wo_sb[:, m, nbs],
                                    rhs=oT[:, m, tbs],
                                    start=(m == 0),
                                    stop=(m == HPC - 1),
                                )
                            grp = (
                                2
                                if (b == B - 1 and tb == NTB_B - 1 and nb >= 12)
                                else 4
                            )
                            if nb % grp == 0:
                                fs = fsb.tile(
                                    [128, 4, TB], F16, tag="fs"
                                )
                            if nb % 2 == 0:
                                nc.vector.tensor_copy(fs[:, nb % grp, :], fo)
                            else:
                                nc.scalar.activation(
                                    fs[:, nb % grp, :], fo, Copy
                                )
                            if nb % grp == grp - 1:
                                last = (
                                    b == B - 1
                                    and tb == NTB_B - 1
                                    and nb == D // 128 - 1
                                )
                                deng = nc.scalar if last else nc.sync
                                deng.dma_start(
                                    out=out_v[
                                        :,
                                        nb - grp + 1 : nb + 1,
                                        t0 + tb * TB : t0 + (tb + 1) * TB,
                                    ],
                                    in_=fs[:, 0:grp, :],
                                )
    _legalize_waits(nc)
    return nc


_NC_CACHE = None


def _get_program():
    global _NC_CACHE
    if _NC_CACHE is None:
        _NC_CACHE = build_program()
    return _NC_CACHE


def _rope_tables():
    inv_freq = 1.0 / (ROPE_THETA ** (np.arange(0, HD, 2, dtype=np.float32) / HD))
    freqs = np.arange(T, dtype=np.float32)[:, None] * inv_freq[None, :]  # (T, 64)
    emb = np.concatenate([freqs, freqs], axis=-1)                        # (T, 128)
    cosT = np.ascontiguousarray(np.cos(emb).T).astype(BF16_NP)           # [128, T]
    sinT = np.sin(emb).T.astype(np.float32)
    sinhT = np.ascontiguousarray(
        np.concatenate([-sinT[: HD // 2], sinT[HD // 2 :]], axis=0)
    ).astype(BF16_NP)
    return cosT, sinhT


def kernel(x, Wq, Wk, Wv, Wo, **run_kwargs):
    x = np.asarray(x, dtype=np.float32)
    Wq = np.asarray(Wq, dtype=np.float32)
    Wk = np.asarray(Wk, dtype=np.float32)
    Wv = np.asarray(Wv, dtype=np.float32)
    Wo = np.asarray(Wo, dtype=np.float32)

    nc = _get_program()
    cosT, sinhT = _rope_tables()
    xT = np.ascontiguousarray(x.reshape(BT, D).T).astype(BF16_NP)  # [D, BT]
    # S^T[tk, tq] causal mask for the diagonal block: keep tq(col) >= tk(row)
    r = np.arange(128)
    negmM = np.where(r[None, :] >= r[:, None], 0.0, -1e30).astype(np.float32)

    in_maps = []
    for c in range(NCORES):
        sl = slice(c * M_PC, (c + 1) * M_PC)
        in_maps.append(
            {
                "xT": xT,
                "negmM": negmM,
                "wqkT": np.ascontiguousarray(
                    np.concatenate([Wq[sl, :].T, Wk[sl, :].T], axis=1)
                ).astype(BF16_NP),
                "wvT": np.ascontiguousarray(Wv[sl, :].T).astype(BF16_NP),
                "woT": np.ascontiguousarray(Wo[:, sl].T).astype(BF16_NP),
                "cosT": cosT,
                "sinhT": sinhT,
            }
        )

    res = run_bass_kernel_spmd(nc, in_maps, list(range(NCORES)), **run_kwargs)
    acc = np.zeros((D, BT), dtype=np.float32)
    for c in range(NCORES):
        acc += res.results[c]["partialT"].astype(np.float32)
    out = np.ascontiguousarray(acc.T).reshape(B, T, D)
    if run_kwargs:
        return out, res
    return out
